# Initial kernel scaffold
#
"""Trainium2 Bass kernel for nn_CrossAttentionFusion.

Reference computation (per batch b):
    xq = F_VNet[b]    viewed as [C=256, N=4096]   (feature-major = native layout)
    xk = F_Knowledge[b] viewed as [32, 4096]
    Q  = Wq @ xq + bq          (feature-major Qt [256, 4096])
    Kt = Wk @ xk  (+bk)        bk is softmax-invariant (constant per query row) -> dropped
    V  = xk.T @ Wv.T (+bv)     bv folds into output bias: bo_eff = bo + Wo @ bv
    St = Kt.T-free attention:  St[k, q] = sum_c Kt[c,k] * (SCALE*Qt[c,q])
    E  = exp(St)               (no max subtraction needed; |S| <~ 15 in fp32)
    Ot[c, q] = sum_k V[k, c] * E[k, q];  d[q] = sum_k E[k, q]  (ones-matmul)
    Yt = (Wo @ Ot) * (1/d) + xq + bo_eff

Sharding: 8 cores = batch(2) x query-chunk(4 x 1024 tokens). K/V replicated
per batch group. Host slices inputs, pre-transposes/folds weights, gathers
output chunks. All device matmuls run in float32r (full PE rate at N>=256).
"""

import os
import sys
import types

import numpy as np

for _p in (
    "/root/.axon_site",
    "/root/.axon_site/_ro/trn_rl_repo",
    "/root/.axon_site/_ro/pypackages",
    "/opt/trn_rl_repo",
):
    if os.path.isdir(_p) and _p not in sys.path:
        sys.path.append(_p)

import concourse.bass as bass  # noqa: E402
import concourse.tile as tile  # noqa: E402
from concourse import bacc, mybir  # noqa: E402
from concourse.bass_utils import run_bass_kernel_spmd  # noqa: E402

F32 = mybir.dt.float32
F32R = mybir.dt.float32r
Act = mybir.ActivationFunctionType
Alu = mybir.AluOpType

B, C, CK = 2, 256, 32
N_TOK = 4096            # H*W*D = 16*16*16, both for queries and keys
QCH = 1024              # query tokens per core
SCALE = (256 // 4) ** (-0.5)
N_CORES = 8

CT = C // 128           # 2 c-tiles of 128
KT = N_TOK // 128       # 32 key tiles of 128
QT = QCH // 512         # 2 query tiles of 512 per core
KB = N_TOK // 512       # 8 key blocks of 512

# Matmul input dtype: float32r runs at full PE rate (1 cyc/row at N>=256)
# with near-fp32 precision. Set KERNEL_MM_F32=1 to fall back to exact fp32.
_MM_DT = F32 if os.environ.get("KERNEL_MM_F32") else F32R


def _r(ap):
    """View an fp32 AP as the matmul input dtype."""
    return ap.bitcast(_MM_DT) if _MM_DT is not F32 else ap


def _install_ntff_hook():
    """Register the axon NTFF profile hook if the image lacks antenv.axon_hooks."""
    try:
        import antenv.axon_hooks  # noqa: F401
        return True
    except ImportError:
        pass
    try:
        import antenv
        mod = types.ModuleType("antenv.axon_hooks")
        _hook = [None]
        mod.set_axon_ntff_profile_hook = lambda h: _hook.__setitem__(0, h)
        mod.get_axon_ntff_profile_hook = lambda: _hook[0]
        sys.modules["antenv.axon_hooks"] = mod
        antenv.axon_hooks = mod
        from trn_agent_boot.trn_boot import _ntff_profile_via_ctypes
        mod.set_axon_ntff_profile_hook(
            _ntff_profile_via_ctypes("/opt/axon/libaxon_pjrt.so")
        )
        return True
    except Exception:
        return False


def _build_program():
    nc = bacc.Bacc(
        "TRN2", target_bir_lowering=False, debug=False, num_devices=N_CORES
    )
    xq = nc.dram_tensor("xq", [C, QCH], F32, kind="ExternalInput").ap()
    xk = nc.dram_tensor("xk", [CK, N_TOK], F32, kind="ExternalInput").ap()
    wq = nc.dram_tensor("wq", [C, C], F32, kind="ExternalInput").ap()   # Wq.T*SCALE
    wk = nc.dram_tensor("wk", [CK, C], F32, kind="ExternalInput").ap()  # Wk.T
    wv = nc.dram_tensor("wv", [CK, C], F32, kind="ExternalInput").ap()  # Wv.T
    wo = nc.dram_tensor("wo", [C, C], F32, kind="ExternalInput").ap()   # Wo.T
    bqs = nc.dram_tensor("bqs", [C], F32, kind="ExternalInput").ap()    # bq*SCALE
    boe = nc.dram_tensor("boe", [C], F32, kind="ExternalInput").ap()    # bo + Wo@bv
    out = nc.dram_tensor("out", [C, QCH], F32, kind="ExternalOutput").ap()
    out_r = out.rearrange("(t p) q -> p t q", p=128)

    with tile.TileContext(nc) as tc:
        with tc.tile_pool(name="singles", bufs=1) as singles:
            xq_sb = singles.tile([128, CT, QCH], F32)
            xk_sb = singles.tile([CK, N_TOK], F32)
            wq_sb = singles.tile([128, CT, C], F32)
            wo_sb = singles.tile([128, CT, C], F32)
            wk_sb = singles.tile([CK, C], F32)
            wv_sb = singles.tile([CK, C], F32)
            bqs_sb = singles.tile([128, CT], F32)
            boe_sb = singles.tile([128, CT], F32)
            ones_sb = singles.tile([128, 128], F32)
            qt_sb = singles.tile([128, CT, QCH], F32)
            kt_sb = singles.tile([128, CT, N_TOK], F32)
            v_sb = singles.tile([128, KT, C], F32)

            nc.sync.dma_start(out=xq_sb, in_=xq.rearrange("(t p) q -> p t q", p=128))
            nc.sync.dma_start(out=xk_sb, in_=xk)
            nc.sync.dma_start(out=wq_sb, in_=wq.rearrange("(t p) n -> p t n", p=128))
            nc.sync.dma_start(out=wo_sb, in_=wo.rearrange("(t p) n -> p t n", p=128))
            nc.sync.dma_start(out=wk_sb, in_=wk)
            nc.sync.dma_start(out=wv_sb, in_=wv)
            nc.sync.dma_start(out=bqs_sb, in_=bqs.rearrange("(t p) -> p t", p=128))
            nc.sync.dma_start(out=boe_sb, in_=boe.rearrange("(t p) -> p t", p=128))
            nc.vector.memset(ones_sb, 1.0)

            # ---- projections ----
            with tc.tile_pool(name="proj_ps", bufs=2, space="PSUM") as pps:
                # Qt[c, q] = wq.T @ xq + bqs (scale folded into wq/bqs on host)
                for qi in range(QT):
                    qsl = slice(qi * 512, (qi + 1) * 512)
                    for ct in range(CT):
                        ps = pps.tile([128, 512], F32, tag="qps")
                        for ci in range(CT):
                            nc.tensor.matmul(
                                ps,
                                lhsT=_r(wq_sb[:, ci, ct * 128:(ct + 1) * 128]),
                                rhs=_r(xq_sb[:, ci, qsl]),
                                start=(ci == 0),
                                stop=(ci == CT - 1),
                            )
                        nc.scalar.activation(
                            out=qt_sb[:, ct, qsl], in_=ps, func=Act.Identity,
                            bias=bqs_sb[:, ct:ct + 1], scale=1.0,
                        )
                # Kt[c, k] = wk.T @ xk (bk softmax-invariant, dropped)
                for ct in range(CT):
                    for kb in range(KB):
                        ps = pps.tile([128, 512], F32, tag="kps")
                        nc.tensor.matmul(
                            ps,
                            lhsT=_r(wk_sb[:, ct * 128:(ct + 1) * 128]),
                            rhs=_r(xk_sb[:, kb * 512:(kb + 1) * 512]),
                        )
                        nc.vector.tensor_copy(kt_sb[:, ct, kb * 512:(kb + 1) * 512], ps)
                # V[k, c] = xk.T @ wv (token-major; bv folded into boe on host)
                for ki in range(KT):
                    ps = pps.tile([128, C], F32, tag="vps")
                    nc.tensor.matmul(
                        ps,
                        lhsT=_r(xk_sb[:, ki * 128:(ki + 1) * 128]),
                        rhs=_r(wv_sb),
                    )
                    nc.vector.tensor_copy(v_sb[:, ki, :], ps)

            # ---- attention (flash-style over k, St layout [k, q]) ----
            with tc.tile_pool(name="s_ps", bufs=3, space="PSUM") as sps, \
                 tc.tile_pool(name="acc_ps", bufs=1, space="PSUM") as aps, \
                 tc.tile_pool(name="y_ps", bufs=1, space="PSUM") as yps, \
                 tc.tile_pool(name="epool", bufs=3) as epool, \
                 tc.tile_pool(name="epi", bufs=2) as epi:
                for qi in range(QT):
                    qsl = slice(qi * 512, (qi + 1) * 512)
                    o_ps = [aps.tile([128, 512], F32, tag=f"o{h}") for h in range(CT)]
                    d_ps = aps.tile([128, 512], F32, tag="d")
                    for ki in range(KT):
                        ksl = slice(ki * 128, (ki + 1) * 128)
                        sp = sps.tile([128, 512], F32, tag="s")
                        for ci in range(CT):
                            nc.tensor.matmul(
                                sp,
                                lhsT=_r(kt_sb[:, ci, ksl]),
                                rhs=_r(qt_sb[:, ci, qsl]),
                                start=(ci == 0),
                                stop=(ci == CT - 1),
                            )
                        e = epool.tile([128, 512], F32, tag="e")
                        nc.scalar.activation(out=e, in_=sp, func=Act.Exp)
                        st, sp_ = (ki == 0), (ki == KT - 1)
                        er = _r(e)
                        for h in range(CT):
                            nc.tensor.matmul(
                                o_ps[h],
                                lhsT=_r(v_sb[:, ki, h * 128:(h + 1) * 128]),
                                rhs=er, start=st, stop=sp_, skip_group_check=True,
                            )
                        nc.tensor.matmul(
                            d_ps, lhsT=_r(ones_sb), rhs=er,
                            start=st, stop=sp_, skip_group_check=True,
                        )
                    # epilogue: rd = 1/d (broadcast across partitions already),
                    # Yt = (wo.T @ Ot) * rd + boe + xq
                    rd = epi.tile([128, 512], F32, tag="rd")
                    nc.vector.reciprocal(rd, d_ps)
                    ob = epi.tile([128, CT, 512], F32, tag="ob")
                    for h in range(CT):
                        nc.vector.tensor_copy(ob[:, h, :], o_ps[h])
                    for co in range(CT):
                        yp = yps.tile([128, 512], F32, tag="y")
                        for ci in range(CT):
                            nc.tensor.matmul(
                                yp,
                                lhsT=_r(wo_sb[:, ci, co * 128:(co + 1) * 128]),
                                rhs=_r(ob[:, ci, :]),
                                start=(ci == 0),
                                stop=(ci == CT - 1),
                            )
                        t = epi.tile([128, 512], F32, tag="t")
                        nc.vector.tensor_mul(t, yp, rd)
                        nc.vector.scalar_tensor_tensor(
                            out=t, in0=t, scalar=boe_sb[:, co:co + 1],
                            in1=xq_sb[:, co, qsl], op0=Alu.add, op1=Alu.add,
                        )
                        nc.sync.dma_start(out=out_r[:, co, qsl], in_=t)

    nc.compile()
    return nc


_NC = None


def _get_nc():
    global _NC
    if _NC is None:
        _NC = _build_program()
    return _NC


def kernel(F_VNet, F_Knowledge, Wq, bq, Wk, bk, Wv, bv, Wo, bo):
    F_VNet = np.asarray(F_VNet, dtype=np.float32)
    F_Knowledge = np.asarray(F_Knowledge, dtype=np.float32)
    Wq, bq = np.asarray(Wq, np.float32), np.asarray(bq, np.float32)
    Wk = np.asarray(Wk, np.float32)
    Wv, bv = np.asarray(Wv, np.float32), np.asarray(bv, np.float32)
    Wo, bo = np.asarray(Wo, np.float32), np.asarray(bo, np.float32)

    in_shape = F_VNet.shape
    xq_full = F_VNet.reshape(B, C, N_TOK)
    xk_full = F_Knowledge.reshape(B, CK, N_TOK)

    wq_h = np.ascontiguousarray(Wq.T * SCALE)
    wk_h = np.ascontiguousarray(Wk.T)
    wv_h = np.ascontiguousarray(Wv.T)
    wo_h = np.ascontiguousarray(Wo.T)
    bqs_h = np.ascontiguousarray(bq * SCALE)
    boe_h = np.ascontiguousarray(bo + Wo @ bv)

    in_maps = []
    for core in range(N_CORES):
        b, j = divmod(core, N_CORES // B)
        in_maps.append({
            "xq": np.ascontiguousarray(xq_full[b, :, j * QCH:(j + 1) * QCH]),
            "xk": np.ascontiguousarray(xk_full[b]),
            "wq": wq_h, "wk": wk_h, "wv": wv_h, "wo": wo_h,
            "bqs": bqs_h, "boe": boe_h,
        })

    trace = bool(os.environ.get("KERNEL_TRACE"))
    if trace:
        _install_ntff_hook()
    nc = _get_nc()
    res = run_bass_kernel_spmd(
        nc, in_maps, core_ids=list(range(N_CORES)), trace=trace
    )
    kernel.last_results = res

    out = np.empty((B, C, N_TOK), np.float32)
    for core in range(N_CORES):
        b, j = divmod(core, N_CORES // B)
        out[b, :, j * QCH:(j + 1) * QCH] = res.results[core]["out"]
    return out.reshape(in_shape)


# revision 10
# speedup vs baseline: 1.1037x; 1.1037x over previous
"""Trainium2 Bass kernel for nn_CrossAttentionFusion.

Math (per batch b), all feature-major on device:
    xq = F_VNet[b]      [C=256, N=4096]   (native layout, no transpose needed)
    xk = F_Knowledge[b] [32, 4096]
    S = Kt.T @ Qt collapses: S = xk.T @ G with G = W_g @ xq + b_g,
        W_g = SCALE*(Wq.T @ Wk).T? -- precisely  G[f,q] = sum_ci wg[ci,f] xq[ci,q],
        wg = SCALE*(Wq.T @ Wk) [256, 32], b_g = SCALE*(Wk.T @ bq) [32].
        (bk is softmax-invariant -> dropped entirely.)
    U  = xk.T @ (Wv.T @ Wo.T)             [Nk, 256]  (Wo folded into V projection;
                                                      bv folds into bo_eff = bo + Wo@bv)
    E = exp(S)   (no max-subtraction: |S| small)
    Yu[co,q] = sum_k U[k,co] E[k,q];  d[q] = sum_k E[k,q]   (ones-matmul, d broadcast
                                                             across partitions by M=128)
    out = Yu * (1/d) + bo_eff + xq

Sharding: 8 cores = batch(2) x query-chunk(4 x 1024 tokens); K/V replicated
within a batch group; host slices inputs / folds weights / gathers outputs.
All matmuls in float32r: measured 227ns issue rate at N=512 (full PE rate)
with ~1e-3 relative precision. A PE warmup burst keeps the HAM clock-gate at
2.4GHz through the DMA-in phase.
"""

import os
import sys
import types

import numpy as np

for _p in (
    "/root/.axon_site",
    "/root/.axon_site/_ro/trn_rl_repo",
    "/root/.axon_site/_ro/pypackages",
    "/opt/trn_rl_repo",
):
    if os.path.isdir(_p) and _p not in sys.path:
        sys.path.append(_p)

import concourse.bass as bass  # noqa: E402,F401
import concourse.tile as tile  # noqa: E402
from concourse import bacc, mybir  # noqa: E402
from concourse.bass_utils import run_bass_kernel_spmd  # noqa: E402

F32 = mybir.dt.float32
F32R = mybir.dt.float32r
Act = mybir.ActivationFunctionType
Alu = mybir.AluOpType

B, C, CK = 2, 256, 32
N_TOK = 4096
QCH = 1024
SCALE = (256 // 4) ** (-0.5)
N_CORES = 8

CT = C // 128           # 2 c-tiles of 128
KT = N_TOK // 128       # 32 key tiles of 128
QT = QCH // 512         # 2 query tiles of 512 per core
KB = N_TOK // 512       # 8 key blocks of 512
N_WARM = int(os.environ.get("KERNEL_WARMUP", "30"))

_MM_DT = F32 if os.environ.get("KERNEL_MM_F32") else F32R


def _install_ntff_hook():
    try:
        import antenv.axon_hooks  # noqa: F401
        return True
    except ImportError:
        pass
    try:
        import antenv
        mod = types.ModuleType("antenv.axon_hooks")
        _hook = [None]
        mod.set_axon_ntff_profile_hook = lambda h: _hook.__setitem__(0, h)
        mod.get_axon_ntff_profile_hook = lambda: _hook[0]
        sys.modules["antenv.axon_hooks"] = mod
        antenv.axon_hooks = mod
        from trn_agent_boot.trn_boot import _ntff_profile_via_ctypes
        mod.set_axon_ntff_profile_hook(
            _ntff_profile_via_ctypes("/opt/axon/libaxon_pjrt.so")
        )
        return True
    except Exception:
        return False


def _build_program():
    nc = bacc.Bacc(
        "TRN2", target_bir_lowering=False, debug=False, num_devices=N_CORES
    )
    MM = _MM_DT
    xq = nc.dram_tensor("xq", [C, QCH], F32, kind="ExternalInput").ap()
    xk = nc.dram_tensor("xk", [CK, N_TOK], F32, kind="ExternalInput").ap()
    wg = nc.dram_tensor("wg", [C, CK], F32, kind="ExternalInput").ap()  # SCALE*Wq.T@Wk
    wu = nc.dram_tensor("wu", [CK, C], F32, kind="ExternalInput").ap()  # Wv.T @ Wo.T
    bg = nc.dram_tensor("bg", [CK], F32, kind="ExternalInput").ap()     # SCALE*Wk.T@bq
    boe = nc.dram_tensor("boe", [C], F32, kind="ExternalInput").ap()    # bo + Wo@bv
    out = nc.dram_tensor("out", [C, QCH], F32, kind="ExternalOutput").ap()
    out_r = out.rearrange("(t p) q -> p t q", p=128)
    xq_r = xq.rearrange("(t p) q -> p t q", p=128).bitcast(MM)
    xk_r = xk.bitcast(MM)

    with tile.TileContext(nc) as tc:
        with tc.tile_pool(name="singles", bufs=1) as singles:
            xq_sb = singles.tile([128, CT, QCH], MM)
            xk_sb = singles.tile([CK, N_TOK], MM)
            wg_sb = singles.tile([128, CT, CK], MM)
            wu_sb = singles.tile([CK, C], MM)
            bg_sb = singles.tile([CK, 1], F32)
            boe_sb = singles.tile([128, CT], F32)
            ones_f = singles.tile([128, 128], F32)
            ones_sb = singles.tile([128, 128], MM)
            g_sb = singles.tile([CK, QCH], MM)
            u_sb = singles.tile([128, KT, C], MM)

            # PE warmup burst: no data deps (memset-fed), keeps the HAM
            # clock-gate busy while input DMAs land.
            nc.vector.memset(ones_f, 1.0)
            nc.vector.tensor_copy(ones_sb, ones_f)
            with tc.tile_pool(name="warm_ps", bufs=1, space="PSUM") as wps:
                wm = wps.tile([128, 128], F32)
                for _ in range(N_WARM):
                    nc.tensor.matmul(
                        wm, lhsT=ones_sb, rhs=ones_sb, start=True, stop=True,
                        skip_group_check=True,
                    )

            # Input DMAs, smallest/most-urgent first; big tensors split so
            # multiple queues run in parallel and consumers unblock early.
            nc.sync.dma_start(out=wu_sb, in_=wu.bitcast(MM))
            nc.sync.dma_start(
                out=wg_sb, in_=wg.rearrange("(t p) f -> p t f", p=128).bitcast(MM)
            )
            nc.sync.dma_start(out=bg_sb, in_=bg[:, None])
            nc.sync.dma_start(out=boe_sb, in_=boe.rearrange("(t p) -> p t", p=128))
            for kb in range(0, KB, 2):
                ks = slice(kb * 512, (kb + 2) * 512)
                nc.sync.dma_start(out=xk_sb[:, ks], in_=xk_r[:, ks])
            for ct in range(CT):
                for qi in range(QT):
                    qsl = slice(qi * 512, (qi + 1) * 512)
                    nc.sync.dma_start(out=xq_sb[:, ct, qsl], in_=xq_r[:, ct, qsl])

            # ---- projections: G = wg.T @ xq + bg;  U = xk.T @ wu ----
            with tc.tile_pool(name="proj_ps", bufs=2, space="PSUM") as pps:
                for qi in range(QT):
                    qsl = slice(qi * 512, (qi + 1) * 512)
                    ps = pps.tile([CK, 512], F32, tag="gps")
                    for ci in range(CT):
                        nc.tensor.matmul(
                            ps,
                            lhsT=wg_sb[:, ci, :],
                            rhs=xq_sb[:, ci, qsl],
                            start=(ci == 0),
                            stop=(ci == CT - 1),
                        )
                    nc.scalar.activation(
                        out=g_sb[:, qsl], in_=ps, func=Act.Identity,
                        bias=bg_sb, scale=1.0,
                    )
                for ki in range(KT):
                    ps = pps.tile([128, C], F32, tag="ups", bufs=4)
                    nc.tensor.matmul(
                        ps,
                        lhsT=xk_sb[:, ki * 128:(ki + 1) * 128],
                        rhs=wu_sb,
                    )
                    nc.vector.tensor_copy(u_sb[:, ki, :], ps)

            # ---- attention (flash over k in St=[k,q] layout) ----
            with tc.tile_pool(name="s_ps", bufs=2, space="PSUM") as sps, \
                 tc.tile_pool(name="acc_ps", bufs=2, space="PSUM") as aps, \
                 tc.tile_pool(name="epool", bufs=6) as epool, \
                 tc.tile_pool(name="epi", bufs=2) as epi:
                for qi in range(QT):
                    qsl = slice(qi * 512, (qi + 1) * 512)
                    y_ps = [
                        aps.tile([128, 512], F32, tag=f"y{h}", name=f"y_ps{h}")
                        for h in range(CT)
                    ]
                    d_ps = aps.tile([128, 512], F32, tag="d")
                    for ki in range(KT):
                        ksl = slice(ki * 128, (ki + 1) * 128)
                        sp = sps.tile([128, 512], F32, tag="s")
                        nc.tensor.matmul(
                            sp, lhsT=xk_sb[:, ksl], rhs=g_sb[:, qsl],
                        )
                        e = epool.tile([128, 512], _MM_DT, tag="e")
                        nc.scalar.activation(out=e, in_=sp, func=Act.Exp)
                        st, fin = (ki == 0), (ki == KT - 1)
                        nc.tensor.matmul(
                            d_ps, lhsT=ones_sb, rhs=e,
                            start=st, stop=fin, skip_group_check=True,
                        )
                        for h in range(CT):
                            nc.tensor.matmul(
                                y_ps[h],
                                lhsT=u_sb[:, ki, h * 128:(h + 1) * 128],
                                rhs=e, start=st, stop=fin, skip_group_check=True,
                            )
                    # epilogue: out = y * (1/d) + boe + xq, in 256-wide halves
                    # so the first multiplies overlap the second reciprocal.
                    rd = epi.tile([128, 512], F32, tag="rd")
                    scr = epi.tile([128, 256], F32, tag="scr")
                    t = [
                        epi.tile([128, 512], F32, tag=f"t{h}", name=f"t{h}")
                        for h in range(CT)
                    ]
                    for half in range(2):
                        hsl = slice(half * 256, (half + 1) * 256)
                        nc.vector.reciprocal_approx_accurate(
                            out=rd[:, hsl], in_=d_ps[:, hsl], scratch=scr
                        )
                        for co in range(CT):
                            nc.vector.tensor_mul(
                                t[co][:, hsl], y_ps[co][:, hsl], rd[:, hsl]
                            )
                    for co in range(CT):
                        nc.vector.scalar_tensor_tensor(
                            out=t[co], in0=t[co], scalar=boe_sb[:, co:co + 1],
                            in1=xq_sb[:, co, qsl].bitcast(F32),
                            op0=Alu.add, op1=Alu.add,
                        )
                        nc.sync.dma_start(out=out_r[:, co, qsl], in_=t[co])

    nc.compile()
    return nc


_NC = None


def _get_nc():
    global _NC
    if _NC is None:
        _NC = _build_program()
    return _NC


def kernel(F_VNet, F_Knowledge, Wq, bq, Wk, bk, Wv, bv, Wo, bo):
    F_VNet = np.asarray(F_VNet, dtype=np.float32)
    F_Knowledge = np.asarray(F_Knowledge, dtype=np.float32)
    Wq, bq = np.asarray(Wq, np.float32), np.asarray(bq, np.float32)
    Wv, bv = np.asarray(Wv, np.float32), np.asarray(bv, np.float32)
    Wk = np.asarray(Wk, np.float32)
    Wo, bo = np.asarray(Wo, np.float32), np.asarray(bo, np.float32)

    in_shape = F_VNet.shape
    xq_full = F_VNet.reshape(B, C, N_TOK)
    xk_full = F_Knowledge.reshape(B, CK, N_TOK)

    wg_h = np.ascontiguousarray(
        (SCALE * Wq.T.astype(np.float64) @ Wk.astype(np.float64)).astype(np.float32)
    )
    wu_h = np.ascontiguousarray(
        (Wv.T.astype(np.float64) @ Wo.T.astype(np.float64)).astype(np.float32)
    )
    bg_h = np.ascontiguousarray(SCALE * (Wk.T @ bq))
    boe_h = np.ascontiguousarray(bo + Wo @ bv)

    in_maps = []
    for core in range(N_CORES):
        b, j = divmod(core, N_CORES // B)
        in_maps.append({
            "xq": np.ascontiguousarray(xq_full[b, :, j * QCH:(j + 1) * QCH]),
            "xk": np.ascontiguousarray(xk_full[b]),
            "wg": wg_h, "wu": wu_h, "bg": bg_h, "boe": boe_h,
        })

    trace = bool(os.environ.get("KERNEL_TRACE"))
    if trace:
        _install_ntff_hook()
    nc = _get_nc()
    res = run_bass_kernel_spmd(
        nc, in_maps, core_ids=list(range(N_CORES)), trace=trace
    )
    kernel.last_results = res

    out = np.empty((B, C, N_TOK), np.float32)
    for core in range(N_CORES):
        b, j = divmod(core, N_CORES // B)
        out[b, :, j * QCH:(j + 1) * QCH] = res.results[core]["out"]
    return out.reshape(in_shape)


# revision 11
# speedup vs baseline: 1.1910x; 1.0791x over previous
"""Trainium2 Bass kernel for nn_CrossAttentionFusion.

Math (per batch b), all feature-major on device:
    xq = F_VNet[b]      [C=256, N=4096]   (native layout, no transpose needed)
    xk = F_Knowledge[b] [32, 4096]
    S = Kt.T @ Qt collapses: S = xk.T @ G with G = W_g @ xq + b_g,
        W_g = SCALE*(Wq.T @ Wk).T? -- precisely  G[f,q] = sum_ci wg[ci,f] xq[ci,q],
        wg = SCALE*(Wq.T @ Wk) [256, 32], b_g = SCALE*(Wk.T @ bq) [32].
        (bk is softmax-invariant -> dropped entirely.)
    U  = xk.T @ (Wv.T @ Wo.T)             [Nk, 256]  (Wo folded into V projection;
                                                      bv folds into bo_eff = bo + Wo@bv)
    E = exp(S)   (no max-subtraction: |S| small)
    Yu[co,q] = sum_k U[k,co] E[k,q];  d[q] = sum_k E[k,q]   (ones-matmul, d broadcast
                                                             across partitions by M=128)
    out = Yu * (1/d) + bo_eff + xq

Sharding: 8 cores = batch(2) x query-chunk(4 x 1024 tokens); K/V replicated
within a batch group; host slices inputs / folds weights / gathers outputs.
All matmuls in float32r: measured 227ns issue rate at N=512 (full PE rate)
with ~1e-3 relative precision. A PE warmup burst keeps the HAM clock-gate at
2.4GHz through the DMA-in phase.
"""

import os
import sys
import types

import numpy as np

for _p in (
    "/root/.axon_site",
    "/root/.axon_site/_ro/trn_rl_repo",
    "/root/.axon_site/_ro/pypackages",
    "/opt/trn_rl_repo",
):
    if os.path.isdir(_p) and _p not in sys.path:
        sys.path.append(_p)

import concourse.bass as bass  # noqa: E402,F401
import concourse.tile as tile  # noqa: E402
from concourse import bacc, mybir  # noqa: E402
from concourse.bass_utils import run_bass_kernel_spmd  # noqa: E402

F32 = mybir.dt.float32
F32R = mybir.dt.float32r
Act = mybir.ActivationFunctionType
Alu = mybir.AluOpType

B, C, CK = 2, 256, 32
N_TOK = 4096
QCH = 1024
SCALE = (256 // 4) ** (-0.5)
N_CORES = 8

CT = C // 128           # 2 c-tiles of 128
KT = N_TOK // 128       # 32 key tiles of 128
QT = QCH // 512         # 2 query tiles of 512 per core
KB = N_TOK // 512       # 8 key blocks of 512
N_WARM = int(os.environ.get("KERNEL_WARMUP", "30"))

_MM_DT = F32 if os.environ.get("KERNEL_MM_F32") else F32R


def _install_ntff_hook():
    try:
        import antenv.axon_hooks  # noqa: F401
        return True
    except ImportError:
        pass
    try:
        import antenv
        mod = types.ModuleType("antenv.axon_hooks")
        _hook = [None]
        mod.set_axon_ntff_profile_hook = lambda h: _hook.__setitem__(0, h)
        mod.get_axon_ntff_profile_hook = lambda: _hook[0]
        sys.modules["antenv.axon_hooks"] = mod
        antenv.axon_hooks = mod
        from trn_agent_boot.trn_boot import _ntff_profile_via_ctypes
        mod.set_axon_ntff_profile_hook(
            _ntff_profile_via_ctypes("/opt/axon/libaxon_pjrt.so")
        )
        return True
    except Exception:
        return False


def _build_program():
    nc = bacc.Bacc(
        "TRN2", target_bir_lowering=False, debug=False, num_devices=N_CORES
    )
    MM = _MM_DT
    xq = nc.dram_tensor("xq", [C, QCH], F32, kind="ExternalInput").ap()
    xk = nc.dram_tensor("xk", [CK, N_TOK], F32, kind="ExternalInput").ap()
    wg = nc.dram_tensor("wg", [C, CK], F32, kind="ExternalInput").ap()  # SCALE*Wq.T@Wk
    wu = nc.dram_tensor("wu", [CK, C], F32, kind="ExternalInput").ap()  # Wv.T @ Wo.T
    bg = nc.dram_tensor("bg", [CK], F32, kind="ExternalInput").ap()     # SCALE*Wk.T@bq
    boe = nc.dram_tensor("boe", [C], F32, kind="ExternalInput").ap()    # bo + Wo@bv
    out = nc.dram_tensor("out", [C, QCH], F32, kind="ExternalOutput").ap()
    out_r = out.rearrange("(t p) q -> p t q", p=128)
    xq_r = xq.rearrange("(t p) q -> p t q", p=128).bitcast(MM)
    xk_r = xk.bitcast(MM)

    with tile.TileContext(nc) as tc:
        with tc.tile_pool(name="singles", bufs=1) as singles:
            xq_sb = singles.tile([128, CT, QCH], MM)
            xk_sb = singles.tile([CK, N_TOK], MM)
            wg_sb = singles.tile([128, CT, CK], MM)
            wu_sb = singles.tile([CK, C], MM)
            bg_sb = singles.tile([CK, 1], F32)
            boe_sb = singles.tile([128, CT], F32)
            ones_f = singles.tile([128, 128], F32)
            ones_sb = singles.tile([128, 128], MM)
            g_sb = singles.tile([CK, QCH], MM)
            u_sb = singles.tile([128, KT, C], MM)

            # PE warmup burst: no data deps (memset-fed), keeps the HAM
            # clock-gate busy while input DMAs land.
            nc.vector.memset(ones_f, 1.0)
            nc.vector.tensor_copy(ones_sb, ones_f)
            with tc.tile_pool(name="warm_ps", bufs=1, space="PSUM") as wps:
                wm = wps.tile([128, 128], F32)
                for _ in range(N_WARM):
                    nc.tensor.matmul(
                        wm, lhsT=ones_sb, rhs=ones_sb, start=True, stop=True,
                        skip_group_check=True,
                    )

            # Input DMAs, smallest/most-urgent first; big tensors split so
            # multiple queues run in parallel and consumers unblock early.
            nc.sync.dma_start(out=wu_sb, in_=wu.bitcast(MM))
            nc.sync.dma_start(
                out=wg_sb, in_=wg.rearrange("(t p) f -> p t f", p=128).bitcast(MM)
            )
            nc.sync.dma_start(out=bg_sb, in_=bg[:, None])
            nc.sync.dma_start(out=boe_sb, in_=boe.rearrange("(t p) -> p t", p=128))
            for kb in range(0, KB, 2):
                ks = slice(kb * 512, (kb + 2) * 512)
                nc.sync.dma_start(out=xk_sb[:, ks], in_=xk_r[:, ks])
            for ct in range(CT):
                for qi in range(QT):
                    qsl = slice(qi * 512, (qi + 1) * 512)
                    nc.sync.dma_start(out=xq_sb[:, ct, qsl], in_=xq_r[:, ct, qsl])

            # ---- projections: G = wg.T @ xq + bg;  U = xk.T @ wu ----
            with tc.tile_pool(name="proj_ps", bufs=2, space="PSUM") as pps:
                for qi in range(QT):
                    qsl = slice(qi * 512, (qi + 1) * 512)
                    ps = pps.tile([CK, 512], F32, tag="gps")
                    for ci in range(CT):
                        nc.tensor.matmul(
                            ps,
                            lhsT=wg_sb[:, ci, :],
                            rhs=xq_sb[:, ci, qsl],
                            start=(ci == 0),
                            stop=(ci == CT - 1),
                        )
                    nc.scalar.activation(
                        out=g_sb[:, qsl], in_=ps, func=Act.Identity,
                        bias=bg_sb, scale=1.0,
                    )
                for ki in range(KT):
                    ps = pps.tile([128, C], F32, tag="ups")
                    nc.tensor.matmul(
                        ps,
                        lhsT=xk_sb[:, ki * 128:(ki + 1) * 128],
                        rhs=wu_sb,
                    )
                    nc.vector.tensor_copy(u_sb[:, ki, :], ps)

            # ---- attention (flash over k in St=[k,q] layout) ----
            with tc.tile_pool(name="s_ps", bufs=3, space="PSUM") as sps, \
                 tc.tile_pool(name="acc_ps", bufs=1, space="PSUM") as aps, \
                 tc.tile_pool(name="epool", bufs=6) as epool, \
                 tc.tile_pool(name="epi", bufs=2) as epi:
                for qi in range(QT):
                    qsl = slice(qi * 512, (qi + 1) * 512)
                    y_ps = [
                        aps.tile([128, 512], F32, tag=f"y{h}", name=f"y_ps{h}")
                        for h in range(CT)
                    ]
                    d_ps = aps.tile([128, 512], F32, tag="d")
                    for ki in range(KT):
                        ksl = slice(ki * 128, (ki + 1) * 128)
                        sp = sps.tile([128, 512], F32, tag="s")
                        nc.tensor.matmul(
                            sp, lhsT=xk_sb[:, ksl], rhs=g_sb[:, qsl],
                        )
                        e = epool.tile([128, 512], _MM_DT, tag="e")
                        nc.scalar.activation(out=e, in_=sp, func=Act.Exp)
                        st, fin = (ki == 0), (ki == KT - 1)
                        nc.tensor.matmul(
                            d_ps, lhsT=ones_sb, rhs=e,
                            start=st, stop=fin, skip_group_check=True,
                        )
                        for h in range(CT):
                            nc.tensor.matmul(
                                y_ps[h],
                                lhsT=u_sb[:, ki, h * 128:(h + 1) * 128],
                                rhs=e, start=st, stop=fin, skip_group_check=True,
                            )
                    # epilogue: out = y * (1/d) + boe + xq, in 256-wide halves
                    # so the first multiplies overlap the second reciprocal.
                    rd = epi.tile([128, 512], F32, tag="rd")
                    scr = epi.tile([128, 256], F32, tag="scr")
                    t = [
                        epi.tile([128, 512], F32, tag=f"t{h}", name=f"t{h}")
                        for h in range(CT)
                    ]
                    for half in range(2):
                        hsl = slice(half * 256, (half + 1) * 256)
                        nc.vector.reciprocal_approx_accurate(
                            out=rd[:, hsl], in_=d_ps[:, hsl], scratch=scr
                        )
                        for co in range(CT):
                            nc.vector.tensor_mul(
                                t[co][:, hsl], y_ps[co][:, hsl], rd[:, hsl]
                            )
                    for co in range(CT):
                        nc.vector.scalar_tensor_tensor(
                            out=t[co], in0=t[co], scalar=boe_sb[:, co:co + 1],
                            in1=xq_sb[:, co, qsl].bitcast(F32),
                            op0=Alu.add, op1=Alu.add,
                        )
                        nc.sync.dma_start(out=out_r[:, co, qsl], in_=t[co])

    nc.compile()
    return nc


_NC = None


def _get_nc():
    global _NC
    if _NC is None:
        _NC = _build_program()
    return _NC


def kernel(F_VNet, F_Knowledge, Wq, bq, Wk, bk, Wv, bv, Wo, bo):
    F_VNet = np.asarray(F_VNet, dtype=np.float32)
    F_Knowledge = np.asarray(F_Knowledge, dtype=np.float32)
    Wq, bq = np.asarray(Wq, np.float32), np.asarray(bq, np.float32)
    Wv, bv = np.asarray(Wv, np.float32), np.asarray(bv, np.float32)
    Wk = np.asarray(Wk, np.float32)
    Wo, bo = np.asarray(Wo, np.float32), np.asarray(bo, np.float32)

    in_shape = F_VNet.shape
    xq_full = F_VNet.reshape(B, C, N_TOK)
    xk_full = F_Knowledge.reshape(B, CK, N_TOK)

    wg_h = np.ascontiguousarray(
        (SCALE * Wq.T.astype(np.float64) @ Wk.astype(np.float64)).astype(np.float32)
    )
    wu_h = np.ascontiguousarray(
        (Wv.T.astype(np.float64) @ Wo.T.astype(np.float64)).astype(np.float32)
    )
    bg_h = np.ascontiguousarray(SCALE * (Wk.T @ bq))
    boe_h = np.ascontiguousarray(bo + Wo @ bv)

    in_maps = []
    for core in range(N_CORES):
        b, j = divmod(core, N_CORES // B)
        in_maps.append({
            "xq": np.ascontiguousarray(xq_full[b, :, j * QCH:(j + 1) * QCH]),
            "xk": np.ascontiguousarray(xk_full[b]),
            "wg": wg_h, "wu": wu_h, "bg": bg_h, "boe": boe_h,
        })

    trace = bool(os.environ.get("KERNEL_TRACE"))
    if trace:
        _install_ntff_hook()
    nc = _get_nc()
    res = run_bass_kernel_spmd(
        nc, in_maps, core_ids=list(range(N_CORES)), trace=trace
    )
    kernel.last_results = res

    out = np.empty((B, C, N_TOK), np.float32)
    for core in range(N_CORES):
        b, j = divmod(core, N_CORES // B)
        out[b, :, j * QCH:(j + 1) * QCH] = res.results[core]["out"]
    return out.reshape(in_shape)


# revision 12
# speedup vs baseline: 1.2905x; 1.0836x over previous
"""Trainium2 Bass kernel for nn_CrossAttentionFusion.

Math (per batch b), all feature-major on device:
    xq = F_VNet[b]      [C=256, N=4096]   (native layout, no transpose needed)
    xk = F_Knowledge[b] [32, 4096]
    S = Kt.T @ Qt collapses: S = xk.T @ G with G = W_g @ xq + b_g,
        W_g = SCALE*(Wq.T @ Wk).T? -- precisely  G[f,q] = sum_ci wg[ci,f] xq[ci,q],
        wg = SCALE*(Wq.T @ Wk) [256, 32], b_g = SCALE*(Wk.T @ bq) [32].
        (bk is softmax-invariant -> dropped entirely.)
    U  = xk.T @ (Wv.T @ Wo.T)             [Nk, 256]  (Wo folded into V projection;
                                                      bv folds into bo_eff = bo + Wo@bv)
    E = exp(S)   (no max-subtraction: |S| small)
    Yu[co,q] = sum_k U[k,co] E[k,q];  d[q] = sum_k E[k,q]   (ones-matmul, d broadcast
                                                             across partitions by M=128)
    out = Yu * (1/d) + bo_eff + xq

Sharding: 8 cores = batch(2) x query-chunk(4 x 1024 tokens); K/V replicated
within a batch group; host slices inputs / folds weights / gathers outputs.
All matmuls in float32r: measured 227ns issue rate at N=512 (full PE rate)
with ~1e-3 relative precision. A PE warmup burst keeps the HAM clock-gate at
2.4GHz through the DMA-in phase.
"""

import os
import sys
import types

import numpy as np

for _p in (
    "/root/.axon_site",
    "/root/.axon_site/_ro/trn_rl_repo",
    "/root/.axon_site/_ro/pypackages",
    "/opt/trn_rl_repo",
):
    if os.path.isdir(_p) and _p not in sys.path:
        sys.path.append(_p)

import concourse.bass as bass  # noqa: E402,F401
import concourse.tile as tile  # noqa: E402
from concourse import bacc, mybir  # noqa: E402
from concourse.bass_utils import run_bass_kernel_spmd  # noqa: E402

F32 = mybir.dt.float32
F32R = mybir.dt.float32r
Act = mybir.ActivationFunctionType
Alu = mybir.AluOpType

B, C, CK = 2, 256, 32
N_TOK = 4096
QCH = 1024
SCALE = (256 // 4) ** (-0.5)
N_CORES = 8

CT = C // 128           # 2 c-tiles of 128
KT = N_TOK // 128       # 32 key tiles of 128
QT = QCH // 512         # 2 query tiles of 512 per core
KB = N_TOK // 512       # 8 key blocks of 512
N_WARM = int(os.environ.get("KERNEL_WARMUP", "30"))

_MM_DT = F32 if os.environ.get("KERNEL_MM_F32") else F32R


def _install_ntff_hook():
    try:
        import antenv.axon_hooks  # noqa: F401
        return True
    except ImportError:
        pass
    try:
        import antenv
        mod = types.ModuleType("antenv.axon_hooks")
        _hook = [None]
        mod.set_axon_ntff_profile_hook = lambda h: _hook.__setitem__(0, h)
        mod.get_axon_ntff_profile_hook = lambda: _hook[0]
        sys.modules["antenv.axon_hooks"] = mod
        antenv.axon_hooks = mod
        from trn_agent_boot.trn_boot import _ntff_profile_via_ctypes
        mod.set_axon_ntff_profile_hook(
            _ntff_profile_via_ctypes("/opt/axon/libaxon_pjrt.so")
        )
        return True
    except Exception:
        return False


def _build_program():
    nc = bacc.Bacc(
        "TRN2", target_bir_lowering=False, debug=False, num_devices=N_CORES
    )
    MM = _MM_DT
    xq = nc.dram_tensor("xq", [C, QCH], F32, kind="ExternalInput").ap()
    xk = nc.dram_tensor("xk", [CK, N_TOK], F32, kind="ExternalInput").ap()
    wg = nc.dram_tensor("wg", [C, CK], F32, kind="ExternalInput").ap()  # SCALE*Wq.T@Wk
    wu = nc.dram_tensor("wu", [CK, C], F32, kind="ExternalInput").ap()  # Wv.T @ Wo.T
    bg = nc.dram_tensor("bg", [CK], F32, kind="ExternalInput").ap()     # SCALE*Wk.T@bq
    boe = nc.dram_tensor("boe", [C], F32, kind="ExternalInput").ap()    # bo + Wo@bv
    out = nc.dram_tensor("out", [C, QCH], F32, kind="ExternalOutput").ap()
    out_r = out.rearrange("(t p) q -> p t q", p=128)
    xq_r = xq.rearrange("(t p) q -> p t q", p=128).bitcast(MM)
    xk_r = xk.bitcast(MM)

    with tile.TileContext(nc) as tc:
        with tc.tile_pool(name="singles", bufs=1) as singles:
            xq_sb = singles.tile([128, CT, QCH], MM)
            xk_sb = singles.tile([CK, N_TOK], MM)
            wg_sb = singles.tile([128, CT, CK], MM)
            wu_sb = singles.tile([CK, C], MM)
            bg_sb = singles.tile([CK, 1], F32)
            boe_sb = singles.tile([128, CT], F32)
            ones_f = singles.tile([128, 128], F32)
            ones_sb = singles.tile([128, 128], MM)
            g_sb = singles.tile([CK, QCH], MM)
            u_sb = singles.tile([128, KT, C], MM)

            # PE warmup burst: no data deps (memset-fed), keeps the HAM
            # clock-gate busy while input DMAs land.
            nc.vector.memset(ones_f, 1.0)
            nc.vector.tensor_copy(ones_sb, ones_f)
            with tc.tile_pool(name="warm_ps", bufs=1, space="PSUM") as wps:
                wm = wps.tile([128, 128], F32)
                for _ in range(N_WARM):
                    nc.tensor.matmul(
                        wm, lhsT=ones_sb, rhs=ones_sb, start=True, stop=True,
                        skip_group_check=True,
                    )

            # Input DMAs, smallest/most-urgent first; big tensors split so
            # multiple queues run in parallel and consumers unblock early.
            nc.sync.dma_start(out=wu_sb, in_=wu.bitcast(MM))
            nc.sync.dma_start(
                out=wg_sb, in_=wg.rearrange("(t p) f -> p t f", p=128).bitcast(MM)
            )
            nc.sync.dma_start(out=bg_sb, in_=bg[:, None])
            nc.sync.dma_start(out=boe_sb, in_=boe.rearrange("(t p) -> p t", p=128))
            for kb in range(0, KB, 2):
                ks = slice(kb * 512, (kb + 2) * 512)
                nc.sync.dma_start(out=xk_sb[:, ks], in_=xk_r[:, ks])
            for ct in range(CT):
                for qi in range(QT):
                    qsl = slice(qi * 512, (qi + 1) * 512)
                    nc.sync.dma_start(out=xq_sb[:, ct, qsl], in_=xq_r[:, ct, qsl])

            # ---- projections: G = wg.T @ xq + bg;  U = xk.T @ wu ----
            with tc.tile_pool(name="proj_ps", bufs=2, space="PSUM") as pps:
                for qi in range(QT):
                    qsl = slice(qi * 512, (qi + 1) * 512)
                    ps = pps.tile([CK, 512], F32, tag="gps")
                    for ci in range(CT):
                        nc.tensor.matmul(
                            ps,
                            lhsT=wg_sb[:, ci, :],
                            rhs=xq_sb[:, ci, qsl],
                            start=(ci == 0),
                            stop=(ci == CT - 1),
                        )
                    nc.scalar.activation(
                        out=g_sb[:, qsl], in_=ps, func=Act.Identity,
                        bias=bg_sb, scale=1.0,
                    )
                for ki in range(KT):
                    ps = pps.tile([128, C], F32, tag="ups", bufs=4)
                    nc.tensor.matmul(
                        ps,
                        lhsT=xk_sb[:, ki * 128:(ki + 1) * 128],
                        rhs=wu_sb,
                    )
                    if ki % 2 == 0:
                        nc.vector.tensor_copy(u_sb[:, ki, :], ps)
                    else:
                        nc.scalar.copy(u_sb[:, ki, :], ps)

            # ---- attention (flash over k in St=[k,q] layout) ----
            with tc.tile_pool(name="s_ps", bufs=4, space="PSUM") as sps, \
                 tc.tile_pool(name="acc_ps", bufs=1, space="PSUM") as aps, \
                 tc.tile_pool(name="epool", bufs=8) as epool, \
                 tc.tile_pool(name="epi", bufs=2) as epi:
                for qi in range(QT):
                    qsl = slice(qi * 512, (qi + 1) * 512)
                    y_ps = [
                        aps.tile([128, 512], F32, tag=f"y{h}", name=f"y_ps{h}")
                        for h in range(CT)
                    ]
                    d_ps = aps.tile([128, 512], F32, tag="d")
                    for ki in range(KT):
                        ksl = slice(ki * 128, (ki + 1) * 128)
                        sp = sps.tile([128, 512], F32, tag="s")
                        nc.tensor.matmul(
                            sp, lhsT=xk_sb[:, ksl], rhs=g_sb[:, qsl],
                        )
                        e = epool.tile([128, 512], _MM_DT, tag="e")
                        nc.scalar.activation(out=e, in_=sp, func=Act.Exp)
                        st, fin = (ki == 0), (ki == KT - 1)
                        nc.tensor.matmul(
                            d_ps, lhsT=ones_sb, rhs=e,
                            start=st, stop=fin, skip_group_check=True,
                        )
                        for h in range(CT):
                            nc.tensor.matmul(
                                y_ps[h],
                                lhsT=u_sb[:, ki, h * 128:(h + 1) * 128],
                                rhs=e, start=st, stop=fin, skip_group_check=True,
                            )
                    # epilogue: out = y * (1/d) + boe + xq, in 256-wide halves
                    # so the first multiplies overlap the second reciprocal.
                    rd = epi.tile([128, 512], F32, tag="rd")
                    scr = epi.tile([128, 256], F32, tag="scr")
                    t = [
                        epi.tile([128, 512], F32, tag=f"t{h}", name=f"t{h}")
                        for h in range(CT)
                    ]
                    for half in range(2):
                        hsl = slice(half * 256, (half + 1) * 256)
                        hslq = slice(qi * 512 + half * 256, qi * 512 + (half + 1) * 256)
                        nc.vector.reciprocal_approx_accurate(
                            out=rd[:, hsl], in_=d_ps[:, hsl], scratch=scr
                        )
                        for co in range(CT):
                            nc.vector.tensor_mul(
                                t[co][:, hsl], y_ps[co][:, hsl], rd[:, hsl]
                            )
                            nc.vector.scalar_tensor_tensor(
                                out=t[co][:, hsl], in0=t[co][:, hsl],
                                scalar=boe_sb[:, co:co + 1],
                                in1=xq_sb[:, co, hslq].bitcast(F32),
                                op0=Alu.add, op1=Alu.add,
                            )
                            nc.sync.dma_start(
                                out=out_r[:, co, hslq], in_=t[co][:, hsl]
                            )

    nc.compile()
    return nc


_NC = None


def _get_nc():
    global _NC
    if _NC is None:
        _NC = _build_program()
    return _NC


def kernel(F_VNet, F_Knowledge, Wq, bq, Wk, bk, Wv, bv, Wo, bo):
    F_VNet = np.asarray(F_VNet, dtype=np.float32)
    F_Knowledge = np.asarray(F_Knowledge, dtype=np.float32)
    Wq, bq = np.asarray(Wq, np.float32), np.asarray(bq, np.float32)
    Wv, bv = np.asarray(Wv, np.float32), np.asarray(bv, np.float32)
    Wk = np.asarray(Wk, np.float32)
    Wo, bo = np.asarray(Wo, np.float32), np.asarray(bo, np.float32)

    in_shape = F_VNet.shape
    xq_full = F_VNet.reshape(B, C, N_TOK)
    xk_full = F_Knowledge.reshape(B, CK, N_TOK)

    wg_h = np.ascontiguousarray(
        (SCALE * Wq.T.astype(np.float64) @ Wk.astype(np.float64)).astype(np.float32)
    )
    wu_h = np.ascontiguousarray(
        (Wv.T.astype(np.float64) @ Wo.T.astype(np.float64)).astype(np.float32)
    )
    bg_h = np.ascontiguousarray(SCALE * (Wk.T @ bq))
    boe_h = np.ascontiguousarray(bo + Wo @ bv)

    in_maps = []
    for core in range(N_CORES):
        b, j = divmod(core, N_CORES // B)
        in_maps.append({
            "xq": np.ascontiguousarray(xq_full[b, :, j * QCH:(j + 1) * QCH]),
            "xk": np.ascontiguousarray(xk_full[b]),
            "wg": wg_h, "wu": wu_h, "bg": bg_h, "boe": boe_h,
        })

    trace = bool(os.environ.get("KERNEL_TRACE"))
    if trace:
        _install_ntff_hook()
    nc = _get_nc()
    res = run_bass_kernel_spmd(
        nc, in_maps, core_ids=list(range(N_CORES)), trace=trace
    )
    kernel.last_results = res

    out = np.empty((B, C, N_TOK), np.float32)
    for core in range(N_CORES):
        b, j = divmod(core, N_CORES // B)
        out[b, :, j * QCH:(j + 1) * QCH] = res.results[core]["out"]
    return out.reshape(in_shape)
